# revision 13
# baseline (speedup 1.0000x reference)
"""PointerNetwork Trainium2 kernel: 8-core data-parallel over batch.

Full inputs -> shard B=2048 across 8 cores (256 rows each) -> per-core Bass
kernel (encoder LSTM, blend1 precompute, 100-step pointer decode) -> gather.
Layout: n' = t*256 + b (t-major); W on partitions for the score pipeline.
"""
import sys, os
sys.path.insert(0, "/opt/trn_rl_repo")
import numpy as np

import concourse.bass as bass
import concourse.mybir as mybir
import concourse.tile as tile
from concourse import bacc
from concourse.bass import ds, ts
from concourse.bass_utils import run_bass_kernel_spmd

P = 128
B = 256          # per-core batch
T = 100
D = 128
H = 256
W = 256
G = 2            # b-groups of 128
HC = 2           # H chunks of 128
WC = 2           # W chunks of 128
MC = 8           # 4H/128 gate chunks
NEG_FILL = -1.0e9
PROB_MIN = 1e-9

WIN = 1024                   # DVE/ACT window = 4 t
TW = WIN // B                # 4 t per window
NWIN = T * B // WIN          # 25 windows per step
NRES = 5                     # resident w-layout windows; rest streamed from DRAM
NWW = 20                     # w-layout windows per step (t < 80)
TBS = 80                     # t-boundary: t >= TBS reduced on DVE in b-layout
FP32 = mybir.dt.float32
U32 = mybir.dt.uint32
AF = mybir.ActivationFunctionType
ALU = mybir.AluOpType
PE = mybir.EngineType.PE

_CACHE = {}


def build_nc(tdec=T, dbg=False):
    nc = bacc.Bacc()
    targT = nc.dram_tensor("targT", [D, T * B], FP32, kind="ExternalInput")
    targflat = nc.dram_tensor("targflat", [B * T, D], FP32, kind="ExternalInput")
    h0T = nc.dram_tensor("h0T", [HC, P, B], FP32, kind="ExternalInput")
    c0T = nc.dram_tensor("c0T", [HC, P, B], FP32, kind="ExternalInput")
    ewih = nc.dram_tensor("ewih", [D, 4 * H], FP32, kind="ExternalInput")
    ewhh = nc.dram_tensor("ewhh", [HC, P, 4 * H], FP32, kind="ExternalInput")
    eb = nc.dram_tensor("eb", [P, MC], FP32, kind="ExternalInput")
    dwih = nc.dram_tensor("dwih", [D, 4 * H], FP32, kind="ExternalInput")
    dwhh = nc.dram_tensor("dwhh", [HC, P, 4 * H], FP32, kind="ExternalInput")
    db = nc.dram_tensor("db", [P, MC], FP32, kind="ExternalInput")
    W1T = nc.dram_tensor("W1T", [HC, P, W], FP32, kind="ExternalInput")
    W2T = nc.dram_tensor("W2T", [HC, P, W], FP32, kind="ExternalInput")
    vte = nc.dram_tensor("vte", [P, WC * 8 * 8], FP32, kind="ExternalInput")
    ident = nc.dram_tensor("ident", [P, P], FP32, kind="ExternalInput")
    iota_t = nc.dram_tensor("iota_t", [P, T], FP32, kind="ExternalInput")
    iota_b = nc.dram_tensor("iota_b", [G, P, 1], U32, kind="ExternalInput")
    consts = nc.dram_tensor("consts", [P, 2], FP32, kind="ExternalInput")
    vtrep = nc.dram_tensor("vtrep", [P, W], FP32, kind="ExternalInput")

    out = nc.dram_tensor("out", [B, T, T], FP32, kind="ExternalOutput")
    if dbg:
        dbg_enc = nc.dram_tensor("dbg_enc", [HC, P, T * B], FP32, kind="ExternalOutput")
        dbg_bl = nc.dram_tensor("dbg_bl", [P, WC, NRES * WIN], FP32, kind="ExternalOutput")
        dbg_sc = nc.dram_tensor("dbg_sc", [P, G, T], FP32, kind="ExternalOutput")
        dbg_b2 = nc.dram_tensor("dbg_b2", [P, WC, B], FP32, kind="ExternalOutput")
        dbg_x = nc.dram_tensor("dbg_x", [P, B], FP32, kind="ExternalOutput")
        dbg_h = nc.dram_tensor("dbg_h", [P, HC, B], FP32, kind="ExternalOutput")

    with tile.TileContext(nc) as tc:
        with tc.tile_pool(name="persist", bufs=1) as pp, \
             tc.tile_pool(name="dramp", bufs=1, space="DRAM") as dmp:
            encd = dmp.tile([HC, P, T * B], FP32)
            bl1d = dmp.tile([WC, P, (NWIN - NRES) * WIN], FP32)
            scsc = dmp.tile([7, 8, 512], FP32)
            wih_t = pp.tile([P, 2, 4 * H], FP32)
            whh_t = pp.tile([P, 2, HC, 4 * H], FP32)
            bias_t = pp.tile([P, 2, MC], FP32)
            w1_t = pp.tile([P, HC, W], FP32)
            w2_t = pp.tile([P, HC, W], FP32)
            vte_t = pp.tile([P, WC, 8, 8], FP32)
            id_t = pp.tile([P, P], FP32)
            iot_t = pp.tile([P, T], FP32)
            iob_t = pp.tile([P, G], U32)
            cst_t = pp.tile([P, 2], FP32)
            h_t = pp.tile([P, HC, B], FP32)
            c_t = pp.tile([P, HC, B], FP32)
            x_t = pp.tile([P, B], FP32)
            mask_t = pp.tile([P, G, T], FP32)
            scores_t = pp.tile([P, G, T], FP32)
            b2_t = pp.tile([P, WC, B], FP32)
            bl1res = pp.tile([P, WC, NRES * WIN], FP32)
            vtr_t = pp.tile([P, W], FP32)
            b2b_t = pp.tile([P, G, W], FP32)
            bl1b = pp.tile([P, G, T - TBS, W], FP32)

            nc.gpsimd.dma_start(wih_t[:, 0, :], ewih[:])
            nc.gpsimd.dma_start(wih_t[:, 1, :], dwih[:])
            nc.gpsimd.dma_start(whh_t[:, 0, :, :], ewhh.rearrange("c p m -> p c m"))
            nc.gpsimd.dma_start(whh_t[:, 1, :, :], dwhh.rearrange("c p m -> p c m"))
            nc.gpsimd.dma_start(bias_t[:, 0, :], eb[:])
            nc.gpsimd.dma_start(bias_t[:, 1, :], db[:])
            nc.gpsimd.dma_start(w1_t[:], W1T.rearrange("c p m -> p c m"))
            nc.gpsimd.dma_start(w2_t[:], W2T.rearrange("c p m -> p c m"))
            nc.gpsimd.dma_start(vte_t[:].rearrange("p c r m -> p (c r m)"), vte[:])
            nc.gpsimd.dma_start(id_t[:], ident[:])
            nc.gpsimd.dma_start(iot_t[:], iota_t[:])
            nc.gpsimd.dma_start(iob_t[:], iota_b.rearrange("g p o -> p (g o)"))
            nc.gpsimd.dma_start(cst_t[:], consts[:])
            nc.gpsimd.dma_start(vtr_t[:], vtrep[:])
            nc.gpsimd.dma_start(h_t[:], h0T.rearrange("c p b -> p c b"))
            nc.gpsimd.dma_start(c_t[:], c0T.rearrange("c p b -> p c b"))
            nc.vector.memset(x_t[:], 0.0)
            nc.vector.memset(mask_t[:], 0.0)

            # ---------------- encoder ----------------
            with tc.tile_pool(name="encp", bufs=3) as ep, \
                 tc.tile_pool(name="encps", bufs=1, space="PSUM") as eps:
                eh_t = pp.tile([P, HC, B], FP32)
                ec_t = pp.tile([P, HC, B], FP32)
                nc.vector.memset(eh_t[:], 0.0)
                nc.vector.memset(ec_t[:], 0.0)
                # 8 gate chunks packed 2-per-bank: 4 psum tiles [128, 512]
                egps = [eps.tile([P, 512], FP32, space="PSUM", name=f"egps{i}", tag=f"egps{i}") for i in range(4)]
                with tc.For_i(0, T, 1, hint_engines=(PE,), name="enc") as t_iv:
                    xw = ep.tile([P, B], FP32, tag="xw")
                    nc.sync.dma_start(xw[:], targT[:, ds(t_iv * B, B)])
                    ga = []
                    for m in range(MC):
                        gp = egps[m // 2][:, (m % 2) * B : (m % 2 + 1) * B]
                        nc.tensor.matmul(gp, wih_t[:, 0, ts(m, P)], xw[:], start=True, stop=False)
                        nc.tensor.matmul(gp, whh_t[:, 0, 0, ts(m, P)], eh_t[:, 0, :], start=False, stop=False)
                        nc.tensor.matmul(gp, whh_t[:, 0, 1, ts(m, P)], eh_t[:, 1, :], start=False, stop=True)
                        g_sb = ep.tile([P, B], FP32, tag=f"gact{m}", name=f"gact{m}")
                        fn = AF.Tanh if m in (4, 5) else AF.Sigmoid
                        nc.scalar.activation(g_sb[:], gp, fn, bias=bias_t[:, 0, m : m + 1])
                        ga.append(g_sb)
                    for hc in range(HC):
                        i_g, f_g, g_g, o_g = ga[hc], ga[2 + hc], ga[4 + hc], ga[6 + hc]
                        tmp = ep.tile([P, B], FP32, tag="tmp")
                        nc.vector.tensor_tensor(out=ec_t[:, hc, :], in0=f_g[:], in1=ec_t[:, hc, :], op=ALU.mult)
                        nc.vector.tensor_tensor(out=tmp[:], in0=i_g[:], in1=g_g[:], op=ALU.mult)
                        nc.vector.tensor_tensor(out=ec_t[:, hc, :], in0=ec_t[:, hc, :], in1=tmp[:], op=ALU.add)
                        thc = ep.tile([P, B], FP32, tag="thc")
                        nc.scalar.activation(thc[:], ec_t[:, hc, :], AF.Tanh)
                        nc.vector.tensor_tensor(out=eh_t[:, hc, :], in0=o_g[:], in1=thc[:], op=ALU.mult)
                        nc.sync.dma_start(encd[hc, :, :][:, ds(t_iv * B, B)], eh_t[:, hc, :])

            # ---------------- blend1 = enc @ W1.T ----------------
            with tc.tile_pool(name="blp", bufs=4) as bp, \
                 tc.tile_pool(name="blps", bufs=4, space="PSUM") as bps:
                for w512 in range(T * B // 512):
                    ew = bp.tile([P, HC, 512], FP32, tag="ew")
                    nc.sync.dma_start(ew[:], encd[:, :, 512 * w512 : 512 * (w512 + 1)].rearrange("c p n -> p c n"))
                    for wc in range(WC):
                        bps_t = bps.tile([P, 512], FP32, space="PSUM", tag="bl")
                        nc.tensor.matmul(bps_t[:], w1_t[:, 0, ts(wc, P)], ew[:, 0, :], start=True, stop=False)
                        nc.tensor.matmul(bps_t[:], w1_t[:, 1, ts(wc, P)], ew[:, 1, :], start=False, stop=True)
                        r = w512 * 512
                        if r + 512 <= NRES * WIN:
                            nc.scalar.activation(bl1res[:, wc, r : r + 512], bps_t[:], AF.Copy)
                        else:
                            bsb = bp.tile([P, 512], FP32, tag="bsb")
                            nc.scalar.activation(bsb[:], bps_t[:], AF.Copy)
                            nc.sync.dma_start(bl1d[wc, :, r - NRES * WIN : r - NRES * WIN + 512], bsb[:])

            # ------- transpose t>=TBS blend1 into b-layout bl1b -------
            with tc.tile_pool(name="tbp", bufs=4) as tbp, \
                 tc.tile_pool(name="tbps", bufs=2, space="PSUM") as tbps:
                base = (NWW - NRES) * WIN  # bl1d col offset of t=TBS
                for wc in range(WC):
                    for ch in range(10):  # 512-col chunks = 2 t each
                        tb_c = tbp.tile([P, 512], FP32, tag="tb_c")
                        nc.sync.dma_start(tb_c[:], bl1d[wc, :, base + 512 * ch : base + 512 * (ch + 1)])
                        for blkk in range(4):
                            t_loc = 2 * ch + blkk // 2
                            g = blkk % 2
                            tpp = tbps.tile([P, P], FP32, space="PSUM", tag="tpp")
                            nc.tensor.transpose(tpp[:], tb_c[:, 128 * blkk : 128 * (blkk + 1)], id_t[:])
                            nc.scalar.activation(bl1b[:, g, t_loc, wc * P : (wc + 1) * P], tpp[:], AF.Copy)

            # ---------------- decode ----------------
            with tc.tile_pool(name="dp", bufs=2) as dp, \
                 tc.tile_pool(name="dps", bufs=1, space="PSUM") as dpsp, \
                 tc.tile_pool(name="dps2", bufs=2, space="PSUM") as dps2:
                dgps = [dpsp.tile([P, 512], FP32, space="PSUM", name=f"dgps{i}", tag=f"dgps{i}") for i in range(4)]
                b2ps = dpsp.tile([P, WC * B], FP32, space="PSUM")
                trps = dpsp.tile([P, B], FP32, space="PSUM")
                with tc.For_i(0, tdec, 1, hint_engines=(PE,), staggered_reset=True, name="dec") as s_iv:
                    # LSTM(dec_in)
                    ga2 = []
                    for m in range(MC):
                        gp = dgps[m // 2][:, (m % 2) * B : (m % 2 + 1) * B]
                        nc.tensor.matmul(gp, wih_t[:, 1, ts(m, P)], x_t[:], start=True, stop=False)
                        nc.tensor.matmul(gp, whh_t[:, 1, 0, ts(m, P)], h_t[:, 0, :], start=False, stop=False)
                        nc.tensor.matmul(gp, whh_t[:, 1, 1, ts(m, P)], h_t[:, 1, :], start=False, stop=True)
                        g_sb = dp.tile([P, B], FP32, tag=f"dgact{m}", name=f"dgact{m}")
                        fn = AF.Tanh if m in (4, 5) else AF.Sigmoid
                        nc.scalar.activation(g_sb[:], gp, fn, bias=bias_t[:, 1, m : m + 1])
                        ga2.append(g_sb)
                    for hc in range(HC):
                        i_g, f_g, g_g, o_g = ga2[hc], ga2[2 + hc], ga2[4 + hc], ga2[6 + hc]
                        tmp = dp.tile([P, B], FP32, tag="dtmp")
                        nc.vector.tensor_tensor(out=c_t[:, hc, :], in0=f_g[:], in1=c_t[:, hc, :], op=ALU.mult)
                        nc.vector.tensor_tensor(out=tmp[:], in0=i_g[:], in1=g_g[:], op=ALU.mult)
                        nc.vector.tensor_tensor(out=c_t[:, hc, :], in0=c_t[:, hc, :], in1=tmp[:], op=ALU.add)
                        thc = dp.tile([P, B], FP32, tag="dthc")
                        nc.scalar.activation(thc[:], c_t[:, hc, :], AF.Tanh)
                        nc.vector.tensor_tensor(out=h_t[:, hc, :], in0=o_g[:], in1=thc[:], op=ALU.mult)
                    # b2 = W2.T h
                    for wc in range(WC):
                        nc.tensor.matmul(b2ps[:, ts(wc, B)], w2_t[:, 0, ts(wc, P)], h_t[:, 0, :], start=True, stop=False)
                        nc.tensor.matmul(b2ps[:, ts(wc, B)], w2_t[:, 1, ts(wc, P)], h_t[:, 1, :], start=False, stop=True)
                    nc.scalar.activation(b2_t[:, 0, :], b2ps[:, 0:B], AF.Copy)
                    nc.scalar.activation(b2_t[:, 1, :], b2ps[:, B : 2 * B], AF.Copy)
                    for wc in range(WC):
                        for g in range(G):
                            nc.tensor.transpose(trps[:, ts(g, P)], b2_t[:, wc, g * P : (g + 1) * P], id_t[:])
                            nc.scalar.activation(b2b_t[:, g, wc * P : (wc + 1) * P], trps[:, ts(g, P)], AF.Copy)

                    # score windows: wp covers t in [4wp, 4wp+4); PE slice = 512 (2 t)
                    scps = None
                    for wp in range(NWW):
                        if wp % 4 == 0:
                            scps = dps2.tile([8, 512], FP32, space="PSUM", tag="scps", name="scps")
                        zs = []
                        for wc in range(WC):
                            arg = dp.tile([P, TW, B], FP32, tag="arg", bufs=2)
                            if wp < NRES:
                                src = bl1res[:, wc, wp * WIN : (wp + 1) * WIN]
                            else:
                                stv = dp.tile([P, WIN], FP32, tag="stv", bufs=2)
                                nc.sync.dma_start(stv[:], bl1d[wc, :, (wp - NRES) * WIN : (wp - NRES + 1) * WIN])
                                src = stv[:]
                            nc.vector.tensor_tensor(
                                out=arg[:], in0=src.rearrange("p (t b) -> p t b", t=TW),
                                in1=b2_t[:, wc, :].unsqueeze(1).to_broadcast([P, TW, B]),
                                op=ALU.add)
                            z = dp.tile([P, TW * B], FP32, tag="z", bufs=2)
                            nc.scalar.activation(z[:], arg[:].rearrange("p t b -> p (t b)"), AF.Tanh)
                            zs.append(z)
                        for half in range(2):
                            row = (2 * wp + half) % 8
                            sl = slice(512 * half, 512 * (half + 1))
                            first = (wp % 4 == 0) and half == 0
                            last = (wp % 4 == 3) and half == 1
                            nc.tensor.matmul(scps[:, :], vte_t[:, 0, row, :], zs[0][:, sl],
                                             start=first, stop=False, skip_group_check=True)
                            nc.tensor.matmul(scps[:, :], vte_t[:, 1, row, :], zs[1][:, sl],
                                             start=False, stop=last, skip_group_check=True)
                        if wp % 4 == 3:
                            blk = wp // 4
                            tb = 4 * (wp - 3)  # 16 t's flushed
                            scf = dp.tile([8, 512], FP32, tag="scf", name="scf")
                            nc.scalar.activation(scf[:], scps[:], AF.Copy)
                            nc.sync.dma_start(scsc[blk, :, :], scf[:])
                            for g in range(G):
                                # dest (b part)(r stride 2t)(u stride 1t); src DRAM strides
                                nc.sync.dma_start(
                                    scores_t[:, g, tb : tb + 16].rearrange("b (r u) -> b r u", u=2),
                                    scsc[blk, :, :].rearrange("r (u g2 b) -> b r u g2", g2=G, b=P)[:, :, :, g],
                                )
                    # b-layout path: t in [TBS, 100) reduced on DVE via stt accum_out
                    for g in range(G):
                        for sub in range(4):
                            tb0 = sub * 5
                            argb = dp.tile([P, 5, W], FP32, tag="argb", bufs=2)
                            nc.vector.tensor_tensor(
                                out=argb[:], in0=bl1b[:, g, tb0 : tb0 + 5, :],
                                in1=b2b_t[:, g, :].unsqueeze(1).to_broadcast([P, 5, W]),
                                op=ALU.add)
                            zb = dp.tile([P, 5 * W], FP32, tag="zb", bufs=2)
                            nc.scalar.activation(zb[:], argb[:].rearrange("p t w -> p (t w)"), AF.Tanh)
                            for tt in range(5):
                                junkb = dp.tile([P, W], FP32, tag="junkb", bufs=4)
                                nc.vector.scalar_tensor_tensor(
                                    out=junkb[:], in0=zb[:, tt * W : (tt + 1) * W], scalar=1.0,
                                    in1=vtr_t[:], op0=ALU.mult, op1=ALU.mult,
                                    accum_out=scores_t[:, g, TBS + tb0 + tt : TBS + tb0 + tt + 1])

                    # mask+argmax+softmax+probs+gather per g
                    nc.vector.tensor_tensor(out=scores_t[:], in0=scores_t[:], in1=mask_t[:], op=ALU.add)
                    for g in range(G):
                        mx = dp.tile([P, 8], FP32, tag=f"mx{g}", name=f"mx{g}")
                        ix = dp.tile([P, 8], U32, tag=f"ix{g}", name=f"ix{g}")
                        nc.vector.max(mx[:], scores_t[:, g, :])
                        nc.vector.max_index(ix[:], mx[:], scores_t[:, g, :])
                        nmx = dp.tile([P, 1], FP32, tag=f"nmx{g}", name=f"nmx{g}")
                        nc.vector.tensor_scalar_mul(nmx[:], mx[:, 0:1], -1.0)
                        ee = dp.tile([P, T], FP32, tag=f"ee{g}", name=f"ee{g}")
                        den = dp.tile([P, 1], FP32, tag=f"den{g}", name=f"den{g}")
                        nc.scalar.activation(ee[:], scores_t[:, g, :], AF.Exp, bias=nmx[:, 0:1], accum_out=den[:])
                        rden = dp.tile([P, 1], FP32, tag=f"rden{g}", name=f"rden{g}")
                        nc.vector.reciprocal(rden[:], den[:])
                        pr = dp.tile([P, T], FP32, tag=f"pr{g}", name=f"pr{g}")
                        nc.vector.scalar_tensor_tensor(
                            out=pr[:], in0=ee[:], scalar=rden[:, 0:1],
                            in1=cst_t[:, 1:2].to_broadcast([P, T]),
                            op0=ALU.mult, op1=ALU.max)
                        nc.sync.dma_start(
                            out[g * P : (g + 1) * P, :, :][:, ds(s_iv, 1), :].rearrange("b o t -> b (o t)"),
                            pr[:])
                        self_f = dp.tile([P, 1], FP32, tag=f"self{g}", name=f"self{g}")
                        nc.vector.tensor_copy(self_f[:], ix[:, 0:1])
                        upd = dp.tile([P, T], FP32, tag=f"upd{g}", name=f"upd{g}")
                        nc.vector.scalar_tensor_tensor(
                            out=upd[:], in0=iot_t[:, :], scalar=self_f[:, 0:1],
                            in1=cst_t[:, 0:1].to_broadcast([P, T]),
                            op0=ALU.is_equal, op1=ALU.mult)
                        nc.vector.tensor_tensor(out=mask_t[:, g, :], in0=mask_t[:, g, :], in1=upd[:], op=ALU.add)
                        gidx = dp.tile([P, 1], U32, tag=f"gidx{g}", name=f"gidx{g}")
                        nc.vector.tensor_tensor(out=gidx[:], in0=iob_t[:, g : g + 1], in1=ix[:, 0:1], op=ALU.add)
                        drow = dp.tile([P, D], FP32, tag=f"drow{g}", name=f"drow{g}")
                        nc.gpsimd.indirect_dma_start(
                            out=drow[:], out_offset=None, in_=targflat[:],
                            in_offset=bass.IndirectOffsetOnAxis(ap=gidx[:, 0:1], axis=0))
                        nc.tensor.transpose(trps[:, ts(g, P)], drow[:], id_t[:])
                        nc.scalar.activation(x_t[:, ts(g, P)], trps[:, ts(g, P)], AF.Copy)

            if dbg:
                nc.sync.dma_start(dbg_enc[:], encd[:])
                nc.sync.dma_start(dbg_bl[:], bl1res[:])
                nc.sync.dma_start(dbg_sc[:], scores_t[:])
                nc.sync.dma_start(dbg_b2[:], b2_t[:])
                nc.sync.dma_start(dbg_x[:], x_t[:])
                nc.sync.dma_start(dbg_h[:], h_t[:])

    nc.finalize()
    return nc


def _const_weights(inp):
    cw = {}
    cw["ewih"] = np.ascontiguousarray(inp["enc_w_ih"].T)
    cw["ewhh"] = np.ascontiguousarray(inp["enc_w_hh"].T.reshape(HC, P, 4 * H))
    cw["eb"] = np.ascontiguousarray((inp["enc_b_ih"] + inp["enc_b_hh"]).reshape(MC, P).T)
    cw["dwih"] = np.ascontiguousarray(inp["dec_w_ih"].T)
    cw["dwhh"] = np.ascontiguousarray(inp["dec_w_hh"].T.reshape(HC, P, 4 * H))
    cw["db"] = np.ascontiguousarray((inp["dec_b_ih"] + inp["dec_b_hh"]).reshape(MC, P).T)
    cw["W1T"] = np.ascontiguousarray(inp["W1"].T.reshape(HC, P, W))
    cw["W2T"] = np.ascontiguousarray(inp["W2"].T.reshape(HC, P, W))
    vte = np.zeros((WC, 8, P, 8), np.float32)
    for wc in range(WC):
        for r in range(8):
            vte[wc, r, :, r] = inp["vt"][wc * P : (wc + 1) * P]
    cw["vte"] = np.ascontiguousarray(vte.transpose(2, 0, 1, 3).reshape(P, WC * 8 * 8))
    cw["ident"] = np.eye(P, dtype=np.float32)
    cw["iota_t"] = np.tile(np.arange(T, dtype=np.float32)[None, :], (P, 1))
    iob = np.zeros((G, P, 1), np.uint32)
    for g in range(G):
        iob[g, :, 0] = (np.arange(P) + g * P) * T
    cw["iota_b"] = iob
    cst = np.zeros((P, 2), np.float32)
    cst[:, 0] = NEG_FILL
    cst[:, 1] = PROB_MIN
    cw["consts"] = cst
    cw["vtrep"] = np.tile(inp["vt"][None, :], (P, 1)).astype(np.float32)
    return cw


def _prep_core(tg, h0, c0, cw):
    i = dict(cw)
    i["targT"] = np.ascontiguousarray(tg.transpose(2, 1, 0).reshape(D, T * B))
    i["targflat"] = np.ascontiguousarray(tg.reshape(B * T, D))
    i["h0T"] = np.ascontiguousarray(h0.T.reshape(HC, P, B))
    i["c0T"] = np.ascontiguousarray(c0.T.reshape(HC, P, B))
    return i


def kernel(**inputs):
    inputs = {k: np.asarray(v, dtype=np.float32) for k, v in inputs.items()}
    if "nc" not in _CACHE:
        _CACHE["nc"] = build_nc()
    nc = _CACHE["nc"]
    cw = _const_weights(inputs)
    tg, h0, c0 = inputs["targets"], inputs["h0"][0], inputs["c0"][0]
    in_maps = [_prep_core(tg[c * B : (c + 1) * B], h0[c * B : (c + 1) * B],
                          c0[c * B : (c + 1) * B], cw) for c in range(8)]
    res = run_bass_kernel_spmd(nc, in_maps, core_ids=list(range(8)))
    return np.concatenate([r["out"] for r in res.results], axis=0)


if __name__ == "__main__":
    build_nc()
    print("build ok")
